# revision 2
# baseline (speedup 1.0000x reference)
"""MinibatchDiscrimination Trainium2 kernel (8-core SPMD), v3.

Computes: M = einsum('nf,fbi->nbi', x, T); l1[n,j,b] = sum_i |M[n,b,i]-M[j,b,i]|;
out = concat([x, sum_j exp(-l1) - 1], axis=1).

Same symmetric-pair sharding as v2: core c gets x row-rotated by -32c; local
row n' pairs with window j = n'+k, k in 1..128; distance-128 dup corrected on
host; mirror (column) contributions accumulated in PSUM and merged host-side.

v3 changes (all cost-model-driven):
 - |d| via X-form: l1 = 2*sum_i max(Mn,Mj) - SM_n - SM_j. The pairwise max is
   computed BATCHED on DVE with overlapping strided views of mt (one
   tensor_tensor(max) per (chunk, n'-block) instead of 32 tensor_scalars),
   2x DVE mode, no per-window scalar needed.
 - Mixed-precision reduction: most chunks stay fp16 (plain E-matmuls); chunks
   routed to GPSIMD/ACT are written as fp8e4 and reduced with DoubleRow
   fp8 matmuls (0.5 cyc/row, two chunks per matmul). GPSIMD/ACT pay no
   dtype penalty, so fp8 conversion is free on those engines.
 - ACT chunks use R-form relu(mt - col) via activation bias; the X/R-form
   correction difference is folded into the host-precomputed Wsumdiff.
 - exp: per-window activation with bias=-SM_n and accum_out row sums.
"""
import sys

sys.path.insert(0, "/opt/trn_rl_repo")

import numpy as np

N = 256       # batch
F = 512       # in features
B = 256       # discrimination features
I = 16        # intermediate features
NCORES = 8
NPER = N // NCORES   # 32 rows per core
KCH = F // 128       # 4 contraction chunks
CCH = (B * I) // 128  # 32 (b,i)-partition chunks
NE = 128 // 8         # 16 distinct E band patterns
W = 128              # pair window (k = 1..128)
JW = NPER + 128      # 160 columns of M needed
NBLK = 8             # n' block size for the batched max
NGRP = NPER // NBLK  # 4 blocks

# --- chunk routing (compile-time tunable) ---------------------------------
# half 0 = chunks 0..15 (b 0..127), half 1 = 16..31 (b 128..255)
POOL_CHUNKS = [13, 14, 15, 29, 30, 31]           # GPSIMD ts, fp8, R-form
ACT_CHUNKS = [10, 11, 26]                        # ACT relu, fp8, R-form
DVE8_CHUNKS = [8, 24]                            # DVE, fp8 (1x tt), X-form
R_FORM_CHUNKS = POOL_CHUNKS + ACT_CHUNKS
DVE_CHUNKS = [c for c in range(CCH)
              if c not in POOL_CHUNKS + ACT_CHUNKS + DVE8_CHUNKS]


def _mk_pairs():
    """Pair fp8 chunks within each half (form-agnostic: the X/R correction
    lives in the host-side Wsumdiff signs). Odd counts get a padded pair
    (second k-tile has zero weights and re-reads the same chunk)."""
    pairs = []
    for h in range(2):
        grp = sorted(c for c in POOL_CHUNKS + DVE8_CHUNKS + ACT_CHUNKS
                     if c // 16 == h)
        for t in range(0, len(grp) - 1, 2):
            pairs.append((grp[t], grp[t + 1]))
        if len(grp) % 2:
            pairs.append((grp[-1], None))
    return pairs


DR_PAIRS = _mk_pairs()
FP8_CHUNKS = sorted(POOL_CHUNKS + ACT_CHUNKS + DVE8_CHUNKS)

_compiled = None


def _build():
    import concourse.bacc as bacc
    import concourse.tile as tile
    from concourse import mybir
    from concourse.ap import AP

    F32 = mybir.dt.float32
    F16 = mybir.dt.float16
    F8 = mybir.dt.float8e4
    DR = mybir.MatmulPerfMode.DoubleRow
    nc = bacc.Bacc(trn_type="TRN2", target_bir_lowering=False)

    xT_d = nc.dram_tensor("xT", [F, JW], F16, kind="ExternalInput")
    wn_d = nc.dram_tensor("Wn", [F, B * I], F16, kind="ExternalInput")
    ws_d = nc.dram_tensor("Wsum", [F, B], F16, kind="ExternalInput")
    wsd_d = nc.dram_tensor("Wsumdiff", [F, B], F16, kind="ExternalInput")
    e16_d = nc.dram_tensor("E16", [NE + 1, 128, 128], F16, kind="ExternalInput")
    e8_d = nc.dram_tensor("E8", [len(DR_PAIRS), 128, 2, 128], F8, kind="ExternalInput")
    row_d = nc.dram_tensor("out_row", [2, 128, NPER], F32, kind="ExternalOutput")
    dup_d = nc.dram_tensor("out_dup", [2, 128, NPER], F16, kind="ExternalOutput")
    col_d = nc.dram_tensor("out_col", [2, 128, JW], F32, kind="ExternalOutput")

    dr_slot = {}
    for s, (c0, c1) in enumerate(DR_PAIRS):
        dr_slot[c0] = (s, 0)
        if c1 is not None:
            dr_slot[c1] = (s, 1)

    with tile.TileContext(nc) as tc:
        with (
            tc.tile_pool(name="wpool", bufs=1) as wpool,
            tc.tile_pool(name="apool", bufs=2) as apool,
            tc.tile_pool(name="epool", bufs=1) as epool,
            tc.tile_pool(name="psmt", bufs=6, space="PSUM") as psmt,
            tc.tile_pool(name="pscol", bufs=2, space="PSUM") as pscol,
        ):
            # ---------------- input DMAs ----------------
            xt_all = wpool.tile([128, KCH, JW], F16, name="xt_all")
            nc.sync.dma_start(xt_all[:], xT_d[:].rearrange("(k p) n -> p k n", k=KCH))
            ws_all = wpool.tile([128, KCH, B], F16, name="ws_all")
            nc.scalar.dma_start(ws_all[:], ws_d[:].rearrange("(k p) b -> p k b", k=KCH))
            wsd_all = wpool.tile([128, KCH, B], F16, name="wsd_all")
            nc.scalar.dma_start(wsd_all[:], wsd_d[:].rearrange("(k p) b -> p k b", k=KCH))
            e_all = wpool.tile([128, NE + 1, 128], F16, name="e_all")
            nc.sync.dma_start(e_all[:], e16_d[:].rearrange("e p q -> p e q"))
            e8_all = wpool.tile([128, len(DR_PAIRS), 2, 128], F8, name="e8_all")
            nc.sync.dma_start(e8_all[:], e8_d[:].rearrange("s p t q -> p s t q"))
            wn_all = wpool.tile([128, KCH, B * I], F16, name="wn_all")
            bounds = [0, 128, 256, 512, 1024, 1536, 2048, 3072, 4096]
            for g in range(len(bounds) - 1):
                lo, hi = bounds[g], bounds[g + 1]
                nc.sync.dma_start(
                    wn_all[:, :, lo:hi],
                    wn_d[:, lo:hi].rearrange("(k p) c -> p k c", k=KCH),
                )

            # ---------------- SM / SMdiff ----------------
            # l1 = sum(2X_c) + sum(2R'_c) - SM[j] - SMdiff[n]
            # (X-form: max(Mj,Mn); R-form: relu(Mj-Mn); SMdiff = SM_X - SM_R)
            # corr matmul moving: smnegd = -SM[j] (full, form-independent)
            # exp bias: sdif_cols = +SMdiff[:, n'] per partition
            sdif_cols = wpool.tile([128, 2, NPER], F32, name="sdif_cols")
            smnegd = wpool.tile([128, 2, JW], F16, name="smnegd")
            for h in range(2):
                ps_sm = psmt.tile([128, JW], F32, name="ps_sm", tag="pt")
                for k in range(KCH):
                    nc.tensor.matmul(
                        ps_sm[:], ws_all[:, k, 128 * h : 128 * (h + 1)],
                        xt_all[:, k, :], start=(k == 0), stop=(k == KCH - 1))
                nc.scalar.mul(out=smnegd[:, h, :], in_=ps_sm[:], mul=-1.0)
                ps_sd = psmt.tile([128, JW], F32, name="ps_sd", tag="pt")
                for k in range(KCH):
                    nc.tensor.matmul(
                        ps_sd[:], wsd_all[:, k, 128 * h : 128 * (h + 1)],
                        xt_all[:, k, :], start=(k == 0), stop=(k == KCH - 1))
                nc.scalar.mul(out=sdif_cols[:, h, :], in_=ps_sd[:, 0:NPER], mul=1.0)

            # ---------------- phase 1: Mt ----------------
            mt = [wpool.tile([128, JW], F16, name=f"mt{c}", tag=f"mt{c}") for c in range(CCH)]
            # negated M columns for ACT-chunk relu bias
            negcols = wpool.tile([128, len(ACT_CHUNKS), NPER], F32, name="negcols")
            poscols = wpool.tile([128, len(POOL_CHUNKS), NPER], F32, name="poscols")
            def _copy(idx, out, in_):
                if idx % 2 == 0:
                    nc.scalar.copy(out=out, in_=in_)
                else:
                    nc.vector.tensor_copy(out, in_)

            for c in range(CCH):
                pt = psmt.tile([128, JW], F32, name="pt", tag="pt")
                for k in range(KCH):
                    nc.tensor.matmul(
                        pt[:], wn_all[:, k, 128 * c : 128 * (c + 1)],
                        xt_all[:, k, :], start=(k == 0), stop=(k == KCH - 1))
                _copy(c, mt[c][:], pt[:])
            for ai, c in enumerate(ACT_CHUNKS):
                nc.vector.tensor_scalar(
                    out=negcols[:, ai, :], in0=mt[c][:, 0:NPER], scalar1=-1.0,
                    scalar2=None, op0=mybir.AluOpType.mult)
            for pi, c in enumerate(POOL_CHUNKS):
                nc.vector.tensor_copy(poscols[:, pi, :], mt[c][:, 0:NPER])

            # ---------------- output accumulators ----------------
            esc_all = [wpool.tile([128, NPER, W], F16, name=f"esc{h}") for h in range(2)]
            row_acc = wpool.tile([128, 2, NPER], F32, name="row_acc")
            zero_pad = wpool.tile([128, JW], F16, name="zero_pad")
            nc.vector.memset(zero_pad[:], 0.0)
            col_ps = [
                pscol.tile([128, JW], F32, name=f"col_ps{h}", tag=f"colps{h}", bufs=1)
                for h in range(2)
            ]
            for h in range(2):
                nc.tensor.matmul(
                    col_ps[h][:], e_all[:, NE, :], zero_pad[:],
                    start=True, stop=False, skip_group_check=True)

            # ---------------- phase 2 ----------------
            for g in range(NGRP):
                base_n = NBLK * g
                # --- batched pairwise terms for this n'-block ---
                a16 = {}
                a8p = {}
                for s, (c0, c1) in enumerate(DR_PAIRS):
                    a8p[s] = apool.tile([128, 2, W, NBLK], F8, name=f"a8_{s}", tag=f"a8_{s}")
                for pi, c in enumerate(POOL_CHUNKS):
                    s, t = dr_slot[c]
                    for nb in range(NBLK):
                        np_ = base_n + nb
                        nc.gpsimd.tensor_scalar(
                            out=a8p[s][:, t, :, nb],
                            in0=mt[c][:, np_ + 1 : np_ + 1 + W],
                            scalar1=poscols[:, pi, np_ : np_ + 1],
                            scalar2=0.0, op0=mybir.AluOpType.subtract,
                            op1=mybir.AluOpType.max)
                for c in DVE8_CHUNKS:
                    s, t = dr_slot[c]
                    mb = mt[c][:]
                    pstr = mb.ap[0][0]
                    in0 = AP(mb.tensor, mb.offset + 1 + base_n, [[pstr, 128], [1, W], [1, NBLK]])
                    in1 = AP(mb.tensor, mb.offset + base_n, [[pstr, 128], [0, W], [1, NBLK]])
                    nc.vector.tensor_tensor(a8p[s][:, t], in0, in1, mybir.AluOpType.max)
                for ai, c in enumerate(ACT_CHUNKS):
                    s, t = dr_slot[c]
                    for nb in range(NBLK):
                        np_ = base_n + nb
                        nc.scalar.activation(
                            out=a8p[s][:, t, :, nb],
                            in_=mt[c][:, np_ + 1 : np_ + 1 + W],
                            func=mybir.ActivationFunctionType.Relu,
                            bias=negcols[:, ai, np_ : np_ + 1], scale=1.0)
                for c in DVE_CHUNKS:
                    a16[c] = apool.tile([128, W, NBLK], F16, name=f"a16_{c}", tag=f"a16_{c}", bufs=2)
                    mb = mt[c][:]
                    pstr = mb.ap[0][0]
                    in0 = AP(mb.tensor, mb.offset + 1 + base_n, [[pstr, 128], [1, W], [1, NBLK]])
                    in1 = AP(mb.tensor, mb.offset + base_n, [[pstr, 128], [0, W], [1, NBLK]])
                    nc.vector.tensor_tensor(a16[c][:], in0, in1, mybir.AluOpType.max)

                # --- windows of this block ---
                for nb in range(NBLK):
                    np_ = base_n + nb
                    off = np_ + 1
                    for h in range(2):
                        ps = psmt.tile([128, W], F32, name="ps", tag="pt")
                        first = True
                        for c in DVE_CHUNKS:
                            if c // 16 != h:
                                continue
                            ab = a16[c][:]
                            mov = AP(ab.tensor, ab.offset + nb,
                                     [[ab.ap[0][0], 128], [NBLK, W]])
                            nc.tensor.matmul(
                                ps[:], e_all[:, c % NE, :], mov,
                                start=first, stop=False)
                            first = False
                        for s, (c0, c1) in enumerate(DR_PAIRS):
                            if c0 // 16 != h:
                                continue
                            ab = a8p[s][:]
                            kst = W * NBLK if c1 is not None else 0
                            mov = AP(ab.tensor, ab.offset + nb,
                                     [[ab.ap[0][0], 128], [kst, 2], [NBLK, W]])
                            nc.tensor.matmul(
                                ps[:], e8_all[:, s], mov,
                                start=first, stop=False, perf_mode=DR)
                            first = False
                        # correction: -(SM_X - SM_R)[j] over the window
                        nc.tensor.matmul(
                            ps[:], e_all[:, NE, :], smnegd[:, h, off : off + W],
                            start=False, stop=True)
                        with tc.high_priority(offset=150):
                            nc.scalar.activation(
                                out=esc_all[h][:, np_, :], in_=ps[:],
                                func=mybir.ActivationFunctionType.Exp,
                                scale=-1.0,
                                bias=sdif_cols[:, h, np_ : np_ + 1],
                                accum_out=row_acc[:, h, np_ : np_ + 1])
                        nc.tensor.matmul(
                            col_ps[h][:, off : off + W], e_all[:, NE, :],
                            esc_all[h][:, np_, :],
                            start=False, stop=(np_ == NPER - 1),
                            skip_group_check=True)

            # ---------------- outputs ----------------
            for h in range(2):
                col_sb = wpool.tile([128, JW], F32, name=f"col_sb{h}")
                nc.scalar.copy(out=col_sb[:], in_=col_ps[h][:])
                nc.sync.dma_start(col_d[h], col_sb[:])
                nc.sync.dma_start(row_d[h], row_acc[:, h, :])
                # dup = k=128 column of each window
                nc.sync.dma_start(dup_d[h], esc_all[h][:, :, W - 1])

    nc.finalize()
    return nc


def _get_compiled():
    global _compiled
    if _compiled is None:
        _compiled = _build()
    return _compiled


def _prep_inputs(x, T):
    """Per-core input maps. Core c gets x row-rotated by -NPER*c."""
    import ml_dtypes

    f16 = np.float16
    f8 = ml_dtypes.float8_e4m3fn
    wn = np.ascontiguousarray(T.reshape(F, B * I)).astype(f16)
    wsum = T.reshape(F, B, I).sum(axis=2).astype(f16)
    # Wsumdiff: +wsum for X-form chunks' b-ranges, -wsum for R-form (ACT)
    sign = np.ones((B,), np.float32)
    for c in R_FORM_CHUNKS:
        sign[8 * c : 8 * c + 8] = -1.0
    wsumdiff = (wsum.astype(np.float32) * sign[None, :]).astype(f16)
    e16 = np.zeros((NE + 1, 128, 128), dtype=f16)
    for ei in range(NE):
        for p in range(128):
            e16[ei, p, 8 * ei + p // 16] = 2.0
    e16[NE] = np.eye(128, dtype=f16)
    e8 = np.zeros((len(DR_PAIRS), 128, 2, 128), dtype=f8)
    for s, pair in enumerate(DR_PAIRS):
        for t, c in enumerate(pair):
            if c is None:
                continue
            ei = c % NE
            for p in range(128):
                e8[s, p, t, 8 * ei + p // 16] = 2.0
    in_maps = []
    for c in range(NCORES):
        xr = np.roll(x, -NPER * c, axis=0)
        xT = np.ascontiguousarray(xr.T[:, 0:JW]).astype(f16)
        in_maps.append({
            "xT": xT, "Wn": wn, "Wsum": wsum, "Wsumdiff": wsumdiff,
            "E16": e16, "E8": e8,
        })
    return in_maps


def _assemble(x, results):
    """Combine symmetric-pair partials (same scheme as v2)."""
    out_disc = np.zeros((N, B), dtype=np.float32)
    for c, res in enumerate(results):
        rows = res["out_row"].transpose(2, 0, 1).reshape(NPER, B)
        dups = res["out_dup"].astype(np.float32).transpose(2, 0, 1).reshape(NPER, B)
        out_disc[NPER * c : NPER * (c + 1), :] += rows - dups
        colg = np.zeros((B, N), np.float32)
        colg[0:128, 0:JW] = res["out_col"][0]
        colg[128:256, 0:JW] = res["out_col"][1]
        out_disc += np.roll(colg.T, NPER * c, axis=0)
    return np.concatenate([x.astype(np.float32), out_disc], axis=1)


def kernel_run(x, T, trace=False):
    from concourse.bass_utils import run_bass_kernel_spmd

    nc = _get_compiled()
    in_maps = _prep_inputs(x, T)
    res = run_bass_kernel_spmd(nc, in_maps, core_ids=list(range(NCORES)), trace=trace)
    return _assemble(x, res.results), res


def kernel(x, T):
    out, _ = kernel_run(x, T, trace=False)
    return out


# revision 7
# speedup vs baseline: 1.0072x; 1.0072x over previous
"""MinibatchDiscrimination Trainium2 kernel (8-core SPMD), v3.

Computes: M = einsum('nf,fbi->nbi', x, T); l1[n,j,b] = sum_i |M[n,b,i]-M[j,b,i]|;
out = concat([x, sum_j exp(-l1) - 1], axis=1).

Same symmetric-pair sharding as v2: core c gets x row-rotated by -32c; local
row n' pairs with window j = n'+k, k in 1..128; distance-128 dup corrected on
host; mirror (column) contributions accumulated in PSUM and merged host-side.

v3 changes (all cost-model-driven):
 - |d| via X-form: l1 = 2*sum_i max(Mn,Mj) - SM_n - SM_j. The pairwise max is
   computed BATCHED on DVE with overlapping strided views of mt (one
   tensor_tensor(max) per (chunk, n'-block) instead of 32 tensor_scalars),
   2x DVE mode, no per-window scalar needed.
 - Mixed-precision reduction: most chunks stay fp16 (plain E-matmuls); chunks
   routed to GPSIMD/ACT are written as fp8e4 and reduced with DoubleRow
   fp8 matmuls (0.5 cyc/row, two chunks per matmul). GPSIMD/ACT pay no
   dtype penalty, so fp8 conversion is free on those engines.
 - ACT chunks use R-form relu(mt - col) via activation bias; the X/R-form
   correction difference is folded into the host-precomputed Wsumdiff.
 - exp: per-window activation with bias=-SM_n and accum_out row sums.
"""
import sys

sys.path.insert(0, "/opt/trn_rl_repo")

import numpy as np

N = 256       # batch
F = 512       # in features
B = 256       # discrimination features
I = 16        # intermediate features
NCORES = 8
NPER = N // NCORES   # 32 rows per core
KCH = F // 128       # 4 contraction chunks
CCH = (B * I) // 128  # 32 (b,i)-partition chunks
NE = 128 // 8         # 16 distinct E band patterns
W = 128              # pair window (k = 1..128)
JW = NPER + 128      # 160 columns of M needed
BLOCKS = [8, 8, 8, 4, 4]   # n' block sizes (smaller at the end -> short tail)
NBLK = 8                   # max block size (buffer sizing)

# --- chunk routing (compile-time tunable) ---------------------------------
# half 0 = chunks 0..15 (b 0..127), half 1 = 16..31 (b 128..255)
POOL_CHUNKS = [13, 14, 15, 29, 30, 31]           # GPSIMD ts, fp8, R-form
ACT_CHUNKS = [10, 26]                            # ACT relu, fp8, R-form
DVE8_CHUNKS = []                                 # DVE, fp8 (1x tt), X-form
R_FORM_CHUNKS = POOL_CHUNKS + ACT_CHUNKS
DVE_CHUNKS = [c for c in range(CCH)
              if c not in POOL_CHUNKS + ACT_CHUNKS + DVE8_CHUNKS]


def _mk_pairs():
    """Pair fp8 chunks within each half (form-agnostic: the X/R correction
    lives in the host-side Wsumdiff signs). Odd counts get a padded pair
    (second k-tile has zero weights and re-reads the same chunk)."""
    pairs = []
    for h in range(2):
        grp = sorted(c for c in POOL_CHUNKS + DVE8_CHUNKS + ACT_CHUNKS
                     if c // 16 == h)
        for t in range(0, len(grp) - 1, 2):
            pairs.append((grp[t], grp[t + 1]))
        if len(grp) % 2:
            pairs.append((grp[-1], None))
    return pairs


DR_PAIRS = _mk_pairs()
FP8_CHUNKS = sorted(POOL_CHUNKS + ACT_CHUNKS + DVE8_CHUNKS)

_compiled = None


def _build():
    import concourse.bacc as bacc
    import concourse.tile as tile
    from concourse import mybir
    from concourse.ap import AP

    F32 = mybir.dt.float32
    F16 = mybir.dt.float16
    F8 = mybir.dt.float8e4
    DR = mybir.MatmulPerfMode.DoubleRow
    nc = bacc.Bacc(trn_type="TRN2", target_bir_lowering=False)

    xT_d = nc.dram_tensor("xT", [F, JW], F16, kind="ExternalInput")
    wn_d = nc.dram_tensor("Wn", [F, B * I], F16, kind="ExternalInput")
    ws_d = nc.dram_tensor("Wsum", [F, B], F16, kind="ExternalInput")
    wsd_d = nc.dram_tensor("Wsumdiff", [F, B], F16, kind="ExternalInput")
    e16_d = nc.dram_tensor("E16", [NE + 1, 128, 128], F16, kind="ExternalInput")
    e8_d = nc.dram_tensor("E8", [len(DR_PAIRS), 128, 2, 128], F8, kind="ExternalInput")
    row_d = nc.dram_tensor("out_row", [2, 128, NPER], F32, kind="ExternalOutput")
    dup_d = nc.dram_tensor("out_dup", [2, 128, NPER], F16, kind="ExternalOutput")
    col_d = nc.dram_tensor("out_col", [2, 128, JW], F32, kind="ExternalOutput")

    dr_slot = {}
    for s, (c0, c1) in enumerate(DR_PAIRS):
        dr_slot[c0] = (s, 0)
        if c1 is not None:
            dr_slot[c1] = (s, 1)

    with tile.TileContext(nc) as tc:
        with (
            tc.tile_pool(name="wpool", bufs=1) as wpool,
            tc.tile_pool(name="apool", bufs=2) as apool,
            tc.tile_pool(name="epool", bufs=1) as epool,
            tc.tile_pool(name="psmt", bufs=6, space="PSUM") as psmt,
            tc.tile_pool(name="pscol", bufs=2, space="PSUM") as pscol,
        ):
            # ---------------- input DMAs ----------------
            xt_all = wpool.tile([128, KCH, JW], F16, name="xt_all")
            nc.sync.dma_start(xt_all[:], xT_d[:].rearrange("(k p) n -> p k n", k=KCH))
            ws_all = wpool.tile([128, KCH, B], F16, name="ws_all")
            nc.scalar.dma_start(ws_all[:], ws_d[:].rearrange("(k p) b -> p k b", k=KCH))
            wsd_all = wpool.tile([128, KCH, B], F16, name="wsd_all")
            nc.scalar.dma_start(wsd_all[:], wsd_d[:].rearrange("(k p) b -> p k b", k=KCH))
            e_all = wpool.tile([128, NE + 1, 128], F16, name="e_all")
            nc.sync.dma_start(e_all[:], e16_d[:].rearrange("e p q -> p e q"))
            e8_all = wpool.tile([128, len(DR_PAIRS), 2, 128], F8, name="e8_all")
            nc.sync.dma_start(e8_all[:], e8_d[:].rearrange("s p t q -> p s t q"))
            wn_all = wpool.tile([128, KCH, B * I], F16, name="wn_all")
            bounds = [0, 128, 256, 512, 1024, 1536, 2048, 3072, 4096]
            for g in range(len(bounds) - 1):
                lo, hi = bounds[g], bounds[g + 1]
                nc.sync.dma_start(
                    wn_all[:, :, lo:hi],
                    wn_d[:, lo:hi].rearrange("(k p) c -> p k c", k=KCH),
                )

            # ---------------- SM / SMdiff ----------------
            # l1 = sum(2X_c) + sum(2R'_c) - SM[j] - SMdiff[n]
            # (X-form: max(Mj,Mn); R-form: relu(Mj-Mn); SMdiff = SM_X - SM_R)
            # corr matmul moving: smnegd = -SM[j] (full, form-independent)
            # exp bias: sdif_cols = +SMdiff[:, n'] per partition
            sdif_cols = wpool.tile([128, 2, NPER], F32, name="sdif_cols")
            smnegd = wpool.tile([128, 2, JW], F16, name="smnegd")
            for h in range(2):
                ps_sm = psmt.tile([128, JW], F32, name="ps_sm", tag="pt")
                for k in range(KCH):
                    nc.tensor.matmul(
                        ps_sm[:], ws_all[:, k, 128 * h : 128 * (h + 1)],
                        xt_all[:, k, :], start=(k == 0), stop=(k == KCH - 1))
                nc.scalar.mul(out=smnegd[:, h, :], in_=ps_sm[:], mul=-1.0)
                ps_sd = psmt.tile([128, JW], F32, name="ps_sd", tag="pt")
                for k in range(KCH):
                    nc.tensor.matmul(
                        ps_sd[:], wsd_all[:, k, 128 * h : 128 * (h + 1)],
                        xt_all[:, k, :], start=(k == 0), stop=(k == KCH - 1))
                nc.scalar.mul(out=sdif_cols[:, h, :], in_=ps_sd[:, 0:NPER], mul=1.0)

            # ---------------- phase 1: Mt ----------------
            mt = [wpool.tile([128, JW], F16, name=f"mt{c}", tag=f"mt{c}") for c in range(CCH)]
            # negated M columns for ACT-chunk relu bias
            negcols = wpool.tile([128, len(ACT_CHUNKS), NPER], F32, name="negcols")
            poscols = wpool.tile([128, len(POOL_CHUNKS), NPER], F32, name="poscols")
            def _copy(idx, out, in_):
                if idx % 8 in (1, 4, 6):
                    nc.vector.tensor_copy(out, in_)
                else:
                    nc.scalar.copy(out=out, in_=in_)

            for c in range(CCH):
                pt = psmt.tile([128, JW], F32, name="pt", tag="pt")
                for k in range(KCH):
                    nc.tensor.matmul(
                        pt[:], wn_all[:, k, 128 * c : 128 * (c + 1)],
                        xt_all[:, k, :], start=(k == 0), stop=(k == KCH - 1))
                _copy(c, mt[c][:], pt[:])
            for ai, c in enumerate(ACT_CHUNKS):
                nc.vector.tensor_scalar(
                    out=negcols[:, ai, :], in0=mt[c][:, 0:NPER], scalar1=-1.0,
                    scalar2=None, op0=mybir.AluOpType.mult)
            for pi, c in enumerate(POOL_CHUNKS):
                nc.vector.tensor_copy(poscols[:, pi, :], mt[c][:, 0:NPER])

            # ---------------- output accumulators ----------------
            esc_all = [wpool.tile([128, NPER, W], F16, name=f"esc{h}") for h in range(2)]
            row_acc = wpool.tile([128, 2, NPER], F32, name="row_acc")
            zero_pad = wpool.tile([128, JW], F16, name="zero_pad")
            nc.vector.memset(zero_pad[:], 0.0)
            col_ps = [
                pscol.tile([128, JW], F32, name=f"col_ps{h}", tag=f"colps{h}", bufs=1)
                for h in range(2)
            ]
            for h in range(2):
                nc.tensor.matmul(
                    col_ps[h][:], e_all[:, NE, :], zero_pad[:],
                    start=True, stop=False, skip_group_check=True)

            # ---------------- phase 2 ----------------
            base_n = 0
            for g, BL in enumerate(BLOCKS):
                # --- batched pairwise terms for this n'-block ---
                a16 = {}
                a8p = {}
                for s, (c0, c1) in enumerate(DR_PAIRS):
                    a8p[s] = apool.tile([128, 2, W, BL], F8, name=f"a8_{s}", tag=f"a8_{s}")
                for pi, c in enumerate(POOL_CHUNKS):
                    s, t = dr_slot[c]
                    for nb in range(BL):
                        np_ = base_n + nb
                        nc.gpsimd.tensor_scalar(
                            out=a8p[s][:, t, :, nb],
                            in0=mt[c][:, np_ + 1 : np_ + 1 + W],
                            scalar1=poscols[:, pi, np_ : np_ + 1],
                            scalar2=0.0, op0=mybir.AluOpType.subtract,
                            op1=mybir.AluOpType.max)
                for c in DVE8_CHUNKS:
                    s, t = dr_slot[c]
                    mb = mt[c][:]
                    pstr = mb.ap[0][0]
                    in0 = AP(mb.tensor, mb.offset + 1 + base_n, [[pstr, 128], [1, W], [1, BL]])
                    in1 = AP(mb.tensor, mb.offset + base_n, [[pstr, 128], [0, W], [1, BL]])
                    nc.vector.tensor_tensor(a8p[s][:, t], in0, in1, mybir.AluOpType.max)
                for ai, c in enumerate(ACT_CHUNKS):
                    s, t = dr_slot[c]
                    for nb in range(BL):
                        np_ = base_n + nb
                        nc.scalar.activation(
                            out=a8p[s][:, t, :, nb],
                            in_=mt[c][:, np_ + 1 : np_ + 1 + W],
                            func=mybir.ActivationFunctionType.Relu,
                            bias=negcols[:, ai, np_ : np_ + 1], scale=1.0)
                for c in DVE_CHUNKS:
                    a16[c] = apool.tile([128, W, BL], F16, name=f"a16_{c}", tag=f"a16_{c}", bufs=2)
                    mb = mt[c][:]
                    pstr = mb.ap[0][0]
                    in0 = AP(mb.tensor, mb.offset + 1 + base_n, [[pstr, 128], [1, W], [1, BL]])
                    in1 = AP(mb.tensor, mb.offset + base_n, [[pstr, 128], [0, W], [1, BL]])
                    nc.vector.tensor_tensor(a16[c][:], in0, in1, mybir.AluOpType.max)

                # --- windows of this block ---
                for nb in range(BL):
                    np_ = base_n + nb
                    off = np_ + 1
                    for h in range(2):
                        ps = psmt.tile([128, W], F32, name="ps", tag="pt")
                        first = True
                        for c in DVE_CHUNKS:
                            if c // 16 != h:
                                continue
                            ab = a16[c][:]
                            mov = AP(ab.tensor, ab.offset + nb,
                                     [[ab.ap[0][0], 128], [BL, W]])
                            nc.tensor.matmul(
                                ps[:], e_all[:, c % NE, :], mov,
                                start=first, stop=False)
                            first = False
                        for s, (c0, c1) in enumerate(DR_PAIRS):
                            if c0 // 16 != h:
                                continue
                            ab = a8p[s][:]
                            kst = W * BL if c1 is not None else 0
                            mov = AP(ab.tensor, ab.offset + nb,
                                     [[ab.ap[0][0], 128], [kst, 2], [BL, W]])
                            nc.tensor.matmul(
                                ps[:], e8_all[:, s], mov,
                                start=first, stop=False, perf_mode=DR)
                            first = False
                        # correction: -(SM_X - SM_R)[j] over the window
                        nc.tensor.matmul(
                            ps[:], e_all[:, NE, :], smnegd[:, h, off : off + W],
                            start=False, stop=True)
                        with tc.high_priority(offset=150):
                            nc.scalar.activation(
                                out=esc_all[h][:, np_, :], in_=ps[:],
                                func=mybir.ActivationFunctionType.Exp,
                                scale=-1.0,
                                bias=sdif_cols[:, h, np_ : np_ + 1],
                                accum_out=row_acc[:, h, np_ : np_ + 1])
                        nc.tensor.matmul(
                            col_ps[h][:, off : off + W], e_all[:, NE, :],
                            esc_all[h][:, np_, :],
                            start=False, stop=(np_ == NPER - 1),
                            skip_group_check=True)
                base_n += BL

            # ---------------- outputs ----------------
            for h in range(2):
                col_sb = wpool.tile([128, JW], F32, name=f"col_sb{h}")
                nc.scalar.copy(out=col_sb[:], in_=col_ps[h][:])
                nc.sync.dma_start(col_d[h], col_sb[:])
                nc.sync.dma_start(row_d[h], row_acc[:, h, :])
                # dup = k=128 column of each window
                nc.sync.dma_start(dup_d[h], esc_all[h][:, :, W - 1])

    nc.finalize()
    return nc


def _get_compiled():
    global _compiled
    if _compiled is None:
        _compiled = _build()
    return _compiled


def _prep_inputs(x, T):
    """Per-core input maps. Core c gets x row-rotated by -NPER*c."""
    import ml_dtypes

    f16 = np.float16
    f8 = ml_dtypes.float8_e4m3fn
    wn = np.ascontiguousarray(T.reshape(F, B * I)).astype(f16)
    wsum = T.reshape(F, B, I).sum(axis=2).astype(f16)
    # Wsumdiff: +wsum for X-form chunks' b-ranges, -wsum for R-form (ACT)
    sign = np.ones((B,), np.float32)
    for c in R_FORM_CHUNKS:
        sign[8 * c : 8 * c + 8] = -1.0
    wsumdiff = (wsum.astype(np.float32) * sign[None, :]).astype(f16)
    e16 = np.zeros((NE + 1, 128, 128), dtype=f16)
    for ei in range(NE):
        for p in range(128):
            e16[ei, p, 8 * ei + p // 16] = 2.0
    e16[NE] = np.eye(128, dtype=f16)
    e8 = np.zeros((len(DR_PAIRS), 128, 2, 128), dtype=f8)
    for s, pair in enumerate(DR_PAIRS):
        for t, c in enumerate(pair):
            if c is None:
                continue
            ei = c % NE
            for p in range(128):
                e8[s, p, t, 8 * ei + p // 16] = 2.0
    in_maps = []
    for c in range(NCORES):
        xr = np.roll(x, -NPER * c, axis=0)
        xT = np.ascontiguousarray(xr.T[:, 0:JW]).astype(f16)
        in_maps.append({
            "xT": xT, "Wn": wn, "Wsum": wsum, "Wsumdiff": wsumdiff,
            "E16": e16, "E8": e8,
        })
    return in_maps


def _assemble(x, results):
    """Combine symmetric-pair partials (same scheme as v2)."""
    out_disc = np.zeros((N, B), dtype=np.float32)
    for c, res in enumerate(results):
        rows = res["out_row"].transpose(2, 0, 1).reshape(NPER, B)
        dups = res["out_dup"].astype(np.float32).transpose(2, 0, 1).reshape(NPER, B)
        out_disc[NPER * c : NPER * (c + 1), :] += rows - dups
        colg = np.zeros((B, N), np.float32)
        colg[0:128, 0:JW] = res["out_col"][0]
        colg[128:256, 0:JW] = res["out_col"][1]
        out_disc += np.roll(colg.T, NPER * c, axis=0)
    return np.concatenate([x.astype(np.float32), out_disc], axis=1)


def kernel_run(x, T, trace=False):
    from concourse.bass_utils import run_bass_kernel_spmd

    nc = _get_compiled()
    in_maps = _prep_inputs(x, T)
    res = run_bass_kernel_spmd(nc, in_maps, core_ids=list(range(NCORES)), trace=trace)
    return _assemble(x, res.results), res


def kernel(x, T):
    out, _ = kernel_run(x, T, trace=False)
    return out


# revision 8
# speedup vs baseline: 1.0118x; 1.0046x over previous
"""MinibatchDiscrimination Trainium2 kernel (8-core SPMD), v3.

Computes: M = einsum('nf,fbi->nbi', x, T); l1[n,j,b] = sum_i |M[n,b,i]-M[j,b,i]|;
out = concat([x, sum_j exp(-l1) - 1], axis=1).

Same symmetric-pair sharding as v2: core c gets x row-rotated by -32c; local
row n' pairs with window j = n'+k, k in 1..128; distance-128 dup corrected on
host; mirror (column) contributions accumulated in PSUM and merged host-side.

v3 changes (all cost-model-driven):
 - |d| via X-form: l1 = 2*sum_i max(Mn,Mj) - SM_n - SM_j. The pairwise max is
   computed BATCHED on DVE with overlapping strided views of mt (one
   tensor_tensor(max) per (chunk, n'-block) instead of 32 tensor_scalars),
   2x DVE mode, no per-window scalar needed.
 - Mixed-precision reduction: most chunks stay fp16 (plain E-matmuls); chunks
   routed to GPSIMD/ACT are written as fp8e4 and reduced with DoubleRow
   fp8 matmuls (0.5 cyc/row, two chunks per matmul). GPSIMD/ACT pay no
   dtype penalty, so fp8 conversion is free on those engines.
 - ACT chunks use R-form relu(mt - col) via activation bias; the X/R-form
   correction difference is folded into the host-precomputed Wsumdiff.
 - exp: per-window activation with bias=-SM_n and accum_out row sums.
"""
import sys

sys.path.insert(0, "/opt/trn_rl_repo")

import numpy as np

N = 256       # batch
F = 512       # in features
B = 256       # discrimination features
I = 16        # intermediate features
NCORES = 8
NPER = N // NCORES   # 32 rows per core
KCH = F // 128       # 4 contraction chunks
CCH = (B * I) // 128  # 32 (b,i)-partition chunks
NE = 128 // 8         # 16 distinct E band patterns
W = 128              # pair window (k = 1..128)
JW = NPER + 128      # 160 columns of M needed
BLOCKS = [8, 8, 8, 4, 4]   # n' block sizes (smaller at the end -> short tail)
NBLK = 8                   # max block size (buffer sizing)

# --- chunk routing (compile-time tunable) ---------------------------------
# half 0 = chunks 0..15 (b 0..127), half 1 = 16..31 (b 128..255)
POOL_CHUNKS = [13, 14, 15, 29, 30, 31]           # GPSIMD ts, fp8, R-form
ACT_CHUNKS = [10, 11, 26, 27]                    # ACT relu, fp8, R-form
DVE8_CHUNKS = []                                 # DVE, fp8 (1x tt), X-form
R_FORM_CHUNKS = POOL_CHUNKS + ACT_CHUNKS
DVE_CHUNKS = [c for c in range(CCH)
              if c not in POOL_CHUNKS + ACT_CHUNKS + DVE8_CHUNKS]


def _mk_pairs():
    """Pair fp8 chunks within each half (form-agnostic: the X/R correction
    lives in the host-side Wsumdiff signs). Odd counts get a padded pair
    (second k-tile has zero weights and re-reads the same chunk)."""
    pairs = []
    for h in range(2):
        grp = sorted(c for c in POOL_CHUNKS + DVE8_CHUNKS + ACT_CHUNKS
                     if c // 16 == h)
        for t in range(0, len(grp) - 1, 2):
            pairs.append((grp[t], grp[t + 1]))
        if len(grp) % 2:
            pairs.append((grp[-1], None))
    return pairs


DR_PAIRS = _mk_pairs()
FP8_CHUNKS = sorted(POOL_CHUNKS + ACT_CHUNKS + DVE8_CHUNKS)

_compiled = None


def _build():
    import concourse.bacc as bacc
    import concourse.tile as tile
    from concourse import mybir
    from concourse.ap import AP

    F32 = mybir.dt.float32
    F16 = mybir.dt.float16
    F8 = mybir.dt.float8e4
    DR = mybir.MatmulPerfMode.DoubleRow
    nc = bacc.Bacc(trn_type="TRN2", target_bir_lowering=False)

    xT_d = nc.dram_tensor("xT", [F, JW], F16, kind="ExternalInput")
    wn_d = nc.dram_tensor("Wn", [F, B * I], F16, kind="ExternalInput")
    ws_d = nc.dram_tensor("Wsum", [F, B], F16, kind="ExternalInput")
    wsd_d = nc.dram_tensor("Wsumdiff", [F, B], F16, kind="ExternalInput")
    e16_d = nc.dram_tensor("E16", [NE + 1, 128, 128], F16, kind="ExternalInput")
    e8_d = nc.dram_tensor("E8", [len(DR_PAIRS), 128, 2, 128], F8, kind="ExternalInput")
    row_d = nc.dram_tensor("out_row", [2, 128, NPER], F32, kind="ExternalOutput")
    dup_d = nc.dram_tensor("out_dup", [2, 128, NPER], F16, kind="ExternalOutput")
    col_d = nc.dram_tensor("out_col", [2, 128, JW], F32, kind="ExternalOutput")

    dr_slot = {}
    for s, (c0, c1) in enumerate(DR_PAIRS):
        dr_slot[c0] = (s, 0)
        if c1 is not None:
            dr_slot[c1] = (s, 1)

    with tile.TileContext(nc) as tc:
        with (
            tc.tile_pool(name="wpool", bufs=1) as wpool,
            tc.tile_pool(name="apool", bufs=2) as apool,
            tc.tile_pool(name="epool", bufs=1) as epool,
            tc.tile_pool(name="psmt", bufs=6, space="PSUM") as psmt,
            tc.tile_pool(name="pscol", bufs=2, space="PSUM") as pscol,
        ):
            # ---------------- input DMAs ----------------
            xt_all = wpool.tile([128, KCH, JW], F16, name="xt_all")
            nc.sync.dma_start(xt_all[:], xT_d[:].rearrange("(k p) n -> p k n", k=KCH))
            ws_all = wpool.tile([128, KCH, B], F16, name="ws_all")
            nc.scalar.dma_start(ws_all[:], ws_d[:].rearrange("(k p) b -> p k b", k=KCH))
            wsd_all = wpool.tile([128, KCH, B], F16, name="wsd_all")
            nc.scalar.dma_start(wsd_all[:], wsd_d[:].rearrange("(k p) b -> p k b", k=KCH))
            e_all = wpool.tile([128, NE + 1, 128], F16, name="e_all")
            nc.sync.dma_start(e_all[:], e16_d[:].rearrange("e p q -> p e q"))
            e8_all = wpool.tile([128, len(DR_PAIRS), 2, 128], F8, name="e8_all")
            nc.sync.dma_start(e8_all[:], e8_d[:].rearrange("s p t q -> p s t q"))
            wn_all = wpool.tile([128, KCH, B * I], F16, name="wn_all")
            bounds = [0, 128, 256, 512, 1024, 1536, 2048, 3072, 4096]
            for g in range(len(bounds) - 1):
                lo, hi = bounds[g], bounds[g + 1]
                nc.sync.dma_start(
                    wn_all[:, :, lo:hi],
                    wn_d[:, lo:hi].rearrange("(k p) c -> p k c", k=KCH),
                )

            # ---------------- SM / SMdiff ----------------
            # l1 = sum(2X_c) + sum(2R'_c) - SM[j] - SMdiff[n]
            # (X-form: max(Mj,Mn); R-form: relu(Mj-Mn); SMdiff = SM_X - SM_R)
            # corr matmul moving: smnegd = -SM[j] (full, form-independent)
            # exp bias: sdif_cols = +SMdiff[:, n'] per partition
            sdif_cols = wpool.tile([128, 2, NPER], F32, name="sdif_cols")
            smnegd = wpool.tile([128, 2, JW], F16, name="smnegd")
            for h in range(2):
                ps_sm = psmt.tile([128, JW], F32, name="ps_sm", tag="pt")
                for k in range(KCH):
                    nc.tensor.matmul(
                        ps_sm[:], ws_all[:, k, 128 * h : 128 * (h + 1)],
                        xt_all[:, k, :], start=(k == 0), stop=(k == KCH - 1))
                nc.scalar.mul(out=smnegd[:, h, :], in_=ps_sm[:], mul=-1.0)
                ps_sd = psmt.tile([128, JW], F32, name="ps_sd", tag="pt")
                for k in range(KCH):
                    nc.tensor.matmul(
                        ps_sd[:], wsd_all[:, k, 128 * h : 128 * (h + 1)],
                        xt_all[:, k, :], start=(k == 0), stop=(k == KCH - 1))
                nc.scalar.mul(out=sdif_cols[:, h, :], in_=ps_sd[:, 0:NPER], mul=1.0)
            sexp = wpool.tile([128, 2, NPER], F32, name="sexp")
            nc.scalar.activation(
                out=sexp[:].rearrange("p a b -> p (a b)"),
                in_=sdif_cols[:].rearrange("p a b -> p (a b)"),
                func=mybir.ActivationFunctionType.Exp, scale=1.0)

            # ---------------- phase 1: Mt ----------------
            mt = [wpool.tile([128, JW], F16, name=f"mt{c}", tag=f"mt{c}") for c in range(CCH)]
            # negated M columns for ACT-chunk relu bias
            negcols = wpool.tile([128, len(ACT_CHUNKS), NPER], F32, name="negcols")
            poscols = wpool.tile([128, len(POOL_CHUNKS), NPER], F32, name="poscols")
            def _copy(idx, out, in_):
                if idx % 8 in (1, 4, 6):
                    nc.vector.tensor_copy(out, in_)
                else:
                    nc.scalar.copy(out=out, in_=in_)

            for c in range(CCH):
                pt = psmt.tile([128, JW], F32, name="pt", tag="pt")
                for k in range(KCH):
                    nc.tensor.matmul(
                        pt[:], wn_all[:, k, 128 * c : 128 * (c + 1)],
                        xt_all[:, k, :], start=(k == 0), stop=(k == KCH - 1))
                _copy(c, mt[c][:], pt[:])
            for ai, c in enumerate(ACT_CHUNKS):
                nc.vector.tensor_scalar(
                    out=negcols[:, ai, :], in0=mt[c][:, 0:NPER], scalar1=-1.0,
                    scalar2=None, op0=mybir.AluOpType.mult)
            for pi, c in enumerate(POOL_CHUNKS):
                nc.vector.tensor_copy(poscols[:, pi, :], mt[c][:, 0:NPER])

            # ---------------- output accumulators ----------------
            esc_all = [wpool.tile([128, NPER, W], F16, name=f"esc{h}") for h in range(2)]
            row_acc = wpool.tile([128, 2, NPER], F32, name="row_acc")
            zero_pad = wpool.tile([128, JW], F16, name="zero_pad")
            nc.vector.memset(zero_pad[:], 0.0)
            col_ps = [
                pscol.tile([128, JW], F32, name=f"col_ps{h}", tag=f"colps{h}", bufs=1)
                for h in range(2)
            ]
            for h in range(2):
                nc.tensor.matmul(
                    col_ps[h][:], e_all[:, NE, :], zero_pad[:],
                    start=True, stop=False, skip_group_check=True)

            # ---------------- phase 2 ----------------
            base_n = 0
            for g, BL in enumerate(BLOCKS):
                # --- batched pairwise terms for this n'-block ---
                a16 = {}
                a8p = {}
                for s, (c0, c1) in enumerate(DR_PAIRS):
                    a8p[s] = apool.tile([128, 2, W, BL], F8, name=f"a8_{s}", tag=f"a8_{s}")
                for pi, c in enumerate(POOL_CHUNKS):
                    s, t = dr_slot[c]
                    for nb in range(BL):
                        np_ = base_n + nb
                        nc.gpsimd.tensor_scalar(
                            out=a8p[s][:, t, :, nb],
                            in0=mt[c][:, np_ + 1 : np_ + 1 + W],
                            scalar1=poscols[:, pi, np_ : np_ + 1],
                            scalar2=0.0, op0=mybir.AluOpType.subtract,
                            op1=mybir.AluOpType.max)
                for c in DVE8_CHUNKS:
                    s, t = dr_slot[c]
                    mb = mt[c][:]
                    pstr = mb.ap[0][0]
                    in0 = AP(mb.tensor, mb.offset + 1 + base_n, [[pstr, 128], [1, W], [1, BL]])
                    in1 = AP(mb.tensor, mb.offset + base_n, [[pstr, 128], [0, W], [1, BL]])
                    nc.vector.tensor_tensor(a8p[s][:, t], in0, in1, mybir.AluOpType.max)
                for ai, c in enumerate(ACT_CHUNKS):
                    s, t = dr_slot[c]
                    for nb in range(BL):
                        np_ = base_n + nb
                        nc.scalar.activation(
                            out=a8p[s][:, t, :, nb],
                            in_=mt[c][:, np_ + 1 : np_ + 1 + W],
                            func=mybir.ActivationFunctionType.Relu,
                            bias=negcols[:, ai, np_ : np_ + 1], scale=1.0)
                for c in DVE_CHUNKS:
                    a16[c] = apool.tile([128, W, BL], F16, name=f"a16_{c}", tag=f"a16_{c}", bufs=2)
                    mb = mt[c][:]
                    pstr = mb.ap[0][0]
                    in0 = AP(mb.tensor, mb.offset + 1 + base_n, [[pstr, 128], [1, W], [1, BL]])
                    in1 = AP(mb.tensor, mb.offset + base_n, [[pstr, 128], [0, W], [1, BL]])
                    nc.vector.tensor_tensor(a16[c][:], in0, in1, mybir.AluOpType.max)

                # --- windows of this block: quads share one psum bank ---
                for q0 in range(0, BL, 4):
                    for h in range(2):
                        ps4 = psmt.tile([128, 4, 128], F32, name="ps4", tag="pt")
                        first = True
                        for qi in range(4):
                            nb = q0 + qi
                            np_ = base_n + nb
                            off = np_ + 1
                            for c in DVE_CHUNKS:
                                if c // 16 != h:
                                    continue
                                ab = a16[c][:]
                                mov = AP(ab.tensor, ab.offset + nb,
                                         [[ab.ap[0][0], 128], [BL, W]])
                                nc.tensor.matmul(
                                    ps4[:, qi, :], e_all[:, c % NE, :], mov,
                                    start=first, stop=False,
                                    skip_group_check=True)
                                first = False
                            for s, (c0, c1) in enumerate(DR_PAIRS):
                                if c0 // 16 != h:
                                    continue
                                ab = a8p[s][:]
                                kst = W * BL if c1 is not None else 0
                                mov = AP(ab.tensor, ab.offset + nb,
                                         [[ab.ap[0][0], 128], [kst, 2], [BL, W]])
                                nc.tensor.matmul(
                                    ps4[:, qi, :], e8_all[:, s], mov,
                                    start=first, stop=False, perf_mode=DR,
                                    skip_group_check=True)
                                first = False
                            nc.tensor.matmul(
                                ps4[:, qi, :], e_all[:, NE, :],
                                smnegd[:, h, off : off + W],
                                start=False,
                                stop=(qi == 3),
                                skip_group_check=True)
                        # one batched exp for 4 windows (no bias)
                        np0 = base_n + q0
                        nc.scalar.activation(
                            out=esc_all[h][:, np0 : np0 + 4, :],
                            in_=ps4[:],
                            func=mybir.ActivationFunctionType.Exp,
                            scale=-1.0)
                        # per-window rescale by e^{SMdiff_n} + row sums
                        for qi in range(4):
                            np_ = np0 + qi
                            off = np_ + 1
                            nc.vector.tensor_scalar(
                                out=esc_all[h][:, np_, :],
                                in0=esc_all[h][:, np_, :],
                                scalar1=sexp[:, h, np_ : np_ + 1],
                                scalar2=None,
                                op0=mybir.AluOpType.mult,
                                op1=mybir.AluOpType.add,
                                accum_out=row_acc[:, h, np_ : np_ + 1])
                            nc.tensor.matmul(
                                col_ps[h][:, off : off + W], e_all[:, NE, :],
                                esc_all[h][:, np_, :],
                                start=False, stop=(np_ == NPER - 1),
                                skip_group_check=True)
                base_n += BL

            # ---------------- outputs ----------------
            for h in range(2):
                col_sb = wpool.tile([128, JW], F32, name=f"col_sb{h}")
                nc.scalar.copy(out=col_sb[:], in_=col_ps[h][:])
                nc.sync.dma_start(col_d[h], col_sb[:])
                nc.sync.dma_start(row_d[h], row_acc[:, h, :])
                # dup = k=128 column of each window
                nc.sync.dma_start(dup_d[h], esc_all[h][:, :, W - 1])

    nc.finalize()
    return nc


def _get_compiled():
    global _compiled
    if _compiled is None:
        _compiled = _build()
    return _compiled


def _prep_inputs(x, T):
    """Per-core input maps. Core c gets x row-rotated by -NPER*c."""
    import ml_dtypes

    f16 = np.float16
    f8 = ml_dtypes.float8_e4m3fn
    wn = np.ascontiguousarray(T.reshape(F, B * I)).astype(f16)
    wsum = T.reshape(F, B, I).sum(axis=2).astype(f16)
    # Wsumdiff: +wsum for X-form chunks' b-ranges, -wsum for R-form (ACT)
    sign = np.ones((B,), np.float32)
    for c in R_FORM_CHUNKS:
        sign[8 * c : 8 * c + 8] = -1.0
    wsumdiff = (wsum.astype(np.float32) * sign[None, :]).astype(f16)
    e16 = np.zeros((NE + 1, 128, 128), dtype=f16)
    for ei in range(NE):
        for p in range(128):
            e16[ei, p, 8 * ei + p // 16] = 2.0
    e16[NE] = np.eye(128, dtype=f16)
    e8 = np.zeros((len(DR_PAIRS), 128, 2, 128), dtype=f8)
    for s, pair in enumerate(DR_PAIRS):
        for t, c in enumerate(pair):
            if c is None:
                continue
            ei = c % NE
            for p in range(128):
                e8[s, p, t, 8 * ei + p // 16] = 2.0
    in_maps = []
    for c in range(NCORES):
        xr = np.roll(x, -NPER * c, axis=0)
        xT = np.ascontiguousarray(xr.T[:, 0:JW]).astype(f16)
        in_maps.append({
            "xT": xT, "Wn": wn, "Wsum": wsum, "Wsumdiff": wsumdiff,
            "E16": e16, "E8": e8,
        })
    return in_maps


def _assemble(x, results):
    """Combine symmetric-pair partials (same scheme as v2)."""
    out_disc = np.zeros((N, B), dtype=np.float32)
    for c, res in enumerate(results):
        rows = res["out_row"].transpose(2, 0, 1).reshape(NPER, B)
        dups = res["out_dup"].astype(np.float32).transpose(2, 0, 1).reshape(NPER, B)
        out_disc[NPER * c : NPER * (c + 1), :] += rows - dups
        colg = np.zeros((B, N), np.float32)
        colg[0:128, 0:JW] = res["out_col"][0]
        colg[128:256, 0:JW] = res["out_col"][1]
        out_disc += np.roll(colg.T, NPER * c, axis=0)
    return np.concatenate([x.astype(np.float32), out_disc], axis=1)


def kernel_run(x, T, trace=False):
    from concourse.bass_utils import run_bass_kernel_spmd

    nc = _get_compiled()
    in_maps = _prep_inputs(x, T)
    res = run_bass_kernel_spmd(nc, in_maps, core_ids=list(range(NCORES)), trace=trace)
    return _assemble(x, res.results), res


def kernel(x, T):
    out, _ = kernel_run(x, T, trace=False)
    return out


# revision 9
# speedup vs baseline: 1.0765x; 1.0639x over previous
"""MinibatchDiscrimination Trainium2 kernel (8-core SPMD), v3.

Computes: M = einsum('nf,fbi->nbi', x, T); l1[n,j,b] = sum_i |M[n,b,i]-M[j,b,i]|;
out = concat([x, sum_j exp(-l1) - 1], axis=1).

Same symmetric-pair sharding as v2: core c gets x row-rotated by -32c; local
row n' pairs with window j = n'+k, k in 1..128; distance-128 dup corrected on
host; mirror (column) contributions accumulated in PSUM and merged host-side.

v3 changes (all cost-model-driven):
 - |d| via X-form: l1 = 2*sum_i max(Mn,Mj) - SM_n - SM_j. The pairwise max is
   computed BATCHED on DVE with overlapping strided views of mt (one
   tensor_tensor(max) per (chunk, n'-block) instead of 32 tensor_scalars),
   2x DVE mode, no per-window scalar needed.
 - Mixed-precision reduction: most chunks stay fp16 (plain E-matmuls); chunks
   routed to GPSIMD/ACT are written as fp8e4 and reduced with DoubleRow
   fp8 matmuls (0.5 cyc/row, two chunks per matmul). GPSIMD/ACT pay no
   dtype penalty, so fp8 conversion is free on those engines.
 - ACT chunks use R-form relu(mt - col) via activation bias; the X/R-form
   correction difference is folded into the host-precomputed Wsumdiff.
 - exp: per-window activation with bias=-SM_n and accum_out row sums.
"""
import sys

sys.path.insert(0, "/opt/trn_rl_repo")

import numpy as np

N = 256       # batch
F = 512       # in features
B = 256       # discrimination features
I = 16        # intermediate features
NCORES = 8
NPER = N // NCORES   # 32 rows per core
KCH = F // 128       # 4 contraction chunks
CCH = (B * I) // 128  # 32 (b,i)-partition chunks
NE = 128 // 8         # 16 distinct E band patterns
W = 128              # pair window (k = 1..128)
JW = NPER + 128      # 160 columns of M needed
BLOCKS = [8, 8, 8, 4, 4]   # n' block sizes (smaller at the end -> short tail)
NBLK = 8                   # max block size (buffer sizing)

# --- chunk routing (compile-time tunable) ---------------------------------
# half 0 = chunks 0..15 (b 0..127), half 1 = 16..31 (b 128..255)
POOL_CHUNKS = [13, 14, 15, 29, 30, 31]           # GPSIMD ts, fp8, R-form
ACT_CHUNKS = [10, 11, 26, 27]                    # ACT relu, fp8, R-form
DVE8_CHUNKS = []                                 # DVE, fp8 (1x tt), X-form
R_FORM_CHUNKS = POOL_CHUNKS + ACT_CHUNKS
DVE_CHUNKS = [c for c in range(CCH)
              if c not in POOL_CHUNKS + ACT_CHUNKS + DVE8_CHUNKS]


def _mk_pairs():
    """Pair fp8 chunks within each half (form-agnostic: the X/R correction
    lives in the host-side Wsumdiff signs). Odd counts get a padded pair
    (second k-tile has zero weights and re-reads the same chunk)."""
    pairs = []
    for h in range(2):
        grp = sorted(c for c in POOL_CHUNKS + DVE8_CHUNKS + ACT_CHUNKS
                     if c // 16 == h)
        for t in range(0, len(grp) - 1, 2):
            pairs.append((grp[t], grp[t + 1]))
        if len(grp) % 2:
            pairs.append((grp[-1], None))
    return pairs


DR_PAIRS = _mk_pairs()
FP8_CHUNKS = sorted(POOL_CHUNKS + ACT_CHUNKS + DVE8_CHUNKS)

_compiled = None


def _build():
    import concourse.bacc as bacc
    import concourse.tile as tile
    from concourse import mybir
    from concourse.ap import AP

    F32 = mybir.dt.float32
    F16 = mybir.dt.float16
    F8 = mybir.dt.float8e4
    DR = mybir.MatmulPerfMode.DoubleRow
    nc = bacc.Bacc(trn_type="TRN2", target_bir_lowering=False)

    xT_d = nc.dram_tensor("xT", [F, JW], F16, kind="ExternalInput")
    wn_d = nc.dram_tensor("Wn", [F, B * I], F16, kind="ExternalInput")
    ws_d = nc.dram_tensor("Wsum", [F, B], F16, kind="ExternalInput")
    wsd_d = nc.dram_tensor("Wsumdiff", [F, B], F16, kind="ExternalInput")
    e16_d = nc.dram_tensor("E16", [NE + 1, 128, 128], F16, kind="ExternalInput")
    e8_d = nc.dram_tensor("E8", [len(DR_PAIRS), 128, 2, 128], F8, kind="ExternalInput")
    row_d = nc.dram_tensor("out_row", [2, 128, NPER], F32, kind="ExternalOutput")
    dup_d = nc.dram_tensor("out_dup", [2, 128, NPER], F16, kind="ExternalOutput")
    col_d = nc.dram_tensor("out_col", [2, 128, JW], F32, kind="ExternalOutput")

    dr_slot = {}
    for s, (c0, c1) in enumerate(DR_PAIRS):
        dr_slot[c0] = (s, 0)
        if c1 is not None:
            dr_slot[c1] = (s, 1)

    with tile.TileContext(nc) as tc:
        with (
            tc.tile_pool(name="wpool", bufs=1) as wpool,
            tc.tile_pool(name="apool", bufs=2) as apool,
            tc.tile_pool(name="epool", bufs=1) as epool,
            tc.tile_pool(name="psmt", bufs=6, space="PSUM") as psmt,
            tc.tile_pool(name="pscol", bufs=2, space="PSUM") as pscol,
        ):
            # ---------------- input DMAs ----------------
            xt_all = wpool.tile([128, KCH, JW], F16, name="xt_all")
            nc.sync.dma_start(xt_all[:], xT_d[:].rearrange("(k p) n -> p k n", k=KCH))
            ws_all = wpool.tile([128, KCH, B], F16, name="ws_all")
            nc.scalar.dma_start(ws_all[:], ws_d[:].rearrange("(k p) b -> p k b", k=KCH))
            wsd_all = wpool.tile([128, KCH, B], F16, name="wsd_all")
            nc.scalar.dma_start(wsd_all[:], wsd_d[:].rearrange("(k p) b -> p k b", k=KCH))
            e_all = wpool.tile([128, NE + 1, 128], F16, name="e_all")
            nc.sync.dma_start(e_all[:], e16_d[:].rearrange("e p q -> p e q"))
            e8_all = wpool.tile([128, len(DR_PAIRS), 2, 128], F8, name="e8_all")
            nc.sync.dma_start(e8_all[:], e8_d[:].rearrange("s p t q -> p s t q"))
            wn_all = wpool.tile([128, KCH, B * I], F16, name="wn_all")
            bounds = [0, 128, 256, 512, 1024, 1536, 2048, 3072, 4096]
            for g in range(len(bounds) - 1):
                lo, hi = bounds[g], bounds[g + 1]
                nc.sync.dma_start(
                    wn_all[:, :, lo:hi],
                    wn_d[:, lo:hi].rearrange("(k p) c -> p k c", k=KCH),
                )

            # ---------------- SM / SMdiff ----------------
            # l1 = sum(2X_c) + sum(2R'_c) - SM[j] - SMdiff[n]
            # (X-form: max(Mj,Mn); R-form: relu(Mj-Mn); SMdiff = SM_X - SM_R)
            # corr matmul moving: smnegd = -SM[j] (full, form-independent)
            # exp bias: sdif_cols = +SMdiff[:, n'] per partition
            sdif_cols = wpool.tile([128, 2, NPER], F32, name="sdif_cols")
            smnegd = wpool.tile([128, 2, JW], F16, name="smnegd")
            for h in range(2):
                ps_sm = psmt.tile([128, JW], F32, name="ps_sm", tag="pt")
                for k in range(KCH):
                    nc.tensor.matmul(
                        ps_sm[:], ws_all[:, k, 128 * h : 128 * (h + 1)],
                        xt_all[:, k, :], start=(k == 0), stop=(k == KCH - 1))
                nc.scalar.mul(out=smnegd[:, h, :], in_=ps_sm[:], mul=-1.0)
                ps_sd = psmt.tile([128, JW], F32, name="ps_sd", tag="pt")
                for k in range(KCH):
                    nc.tensor.matmul(
                        ps_sd[:], wsd_all[:, k, 128 * h : 128 * (h + 1)],
                        xt_all[:, k, :], start=(k == 0), stop=(k == KCH - 1))
                nc.scalar.mul(out=sdif_cols[:, h, :], in_=ps_sd[:, 0:NPER], mul=1.0)
            sexp = wpool.tile([128, 2, NPER], F32, name="sexp")
            nc.scalar.activation(
                out=sexp[:].rearrange("p a b -> p (a b)"),
                in_=sdif_cols[:].rearrange("p a b -> p (a b)"),
                func=mybir.ActivationFunctionType.Exp, scale=1.0)

            # ---------------- phase 1: Mt ----------------
            mt = [wpool.tile([128, JW], F16, name=f"mt{c}", tag=f"mt{c}") for c in range(CCH)]
            # negated M columns for ACT-chunk relu bias
            negcols = wpool.tile([128, len(ACT_CHUNKS), NPER], F32, name="negcols")
            poscols = wpool.tile([128, len(POOL_CHUNKS), NPER], F32, name="poscols")
            def _copy(idx, out, in_):
                nc.scalar.copy(out=out, in_=in_)

            for c in range(CCH):
                pt = psmt.tile([128, JW], F32, name="pt", tag="pt")
                for k in range(KCH):
                    nc.tensor.matmul(
                        pt[:], wn_all[:, k, 128 * c : 128 * (c + 1)],
                        xt_all[:, k, :], start=(k == 0), stop=(k == KCH - 1))
                _copy(c, mt[c][:], pt[:])
            for ai, c in enumerate(ACT_CHUNKS):
                nc.vector.tensor_scalar(
                    out=negcols[:, ai, :], in0=mt[c][:, 0:NPER], scalar1=-1.0,
                    scalar2=None, op0=mybir.AluOpType.mult)
            for pi, c in enumerate(POOL_CHUNKS):
                nc.vector.tensor_copy(poscols[:, pi, :], mt[c][:, 0:NPER])

            # ---------------- output accumulators ----------------
            esc_all = [wpool.tile([128, NPER, W], F16, name=f"esc{h}") for h in range(2)]
            row_acc = wpool.tile([128, 2, NPER], F32, name="row_acc")
            zero_pad = wpool.tile([128, JW], F16, name="zero_pad")
            nc.vector.memset(zero_pad[:], 0.0)
            col_ps = [
                pscol.tile([128, JW], F32, name=f"col_ps{h}", tag=f"colps{h}", bufs=1)
                for h in range(2)
            ]
            for h in range(2):
                nc.tensor.matmul(
                    col_ps[h][:], e_all[:, NE, :], zero_pad[:],
                    start=True, stop=False, skip_group_check=True)

            # ---------------- phase 2 ----------------
            base_n = 0
            for g, BL in enumerate(BLOCKS):
                # --- batched pairwise terms for this n'-block ---
                a16 = {}
                a8p = {}
                for s, (c0, c1) in enumerate(DR_PAIRS):
                    a8p[s] = apool.tile([128, 2, W, BL], F8, name=f"a8_{s}", tag=f"a8_{s}")
                for pi, c in enumerate(POOL_CHUNKS):
                    s, t = dr_slot[c]
                    for nb in range(BL):
                        np_ = base_n + nb
                        nc.gpsimd.tensor_scalar(
                            out=a8p[s][:, t, :, nb],
                            in0=mt[c][:, np_ + 1 : np_ + 1 + W],
                            scalar1=poscols[:, pi, np_ : np_ + 1],
                            scalar2=0.0, op0=mybir.AluOpType.subtract,
                            op1=mybir.AluOpType.max)
                for c in DVE8_CHUNKS:
                    s, t = dr_slot[c]
                    mb = mt[c][:]
                    pstr = mb.ap[0][0]
                    in0 = AP(mb.tensor, mb.offset + 1 + base_n, [[pstr, 128], [1, W], [1, BL]])
                    in1 = AP(mb.tensor, mb.offset + base_n, [[pstr, 128], [0, W], [1, BL]])
                    nc.vector.tensor_tensor(a8p[s][:, t], in0, in1, mybir.AluOpType.max)
                for ai, c in enumerate(ACT_CHUNKS):
                    s, t = dr_slot[c]
                    for nb in range(BL):
                        np_ = base_n + nb
                        nc.scalar.activation(
                            out=a8p[s][:, t, :, nb],
                            in_=mt[c][:, np_ + 1 : np_ + 1 + W],
                            func=mybir.ActivationFunctionType.Relu,
                            bias=negcols[:, ai, np_ : np_ + 1], scale=1.0)
                for c in DVE_CHUNKS:
                    a16[c] = apool.tile([128, W, BL], F16, name=f"a16_{c}", tag=f"a16_{c}", bufs=2)
                    mb = mt[c][:]
                    pstr = mb.ap[0][0]
                    in0 = AP(mb.tensor, mb.offset + 1 + base_n, [[pstr, 128], [1, W], [1, BL]])
                    in1 = AP(mb.tensor, mb.offset + base_n, [[pstr, 128], [0, W], [1, BL]])
                    nc.vector.tensor_tensor(a16[c][:], in0, in1, mybir.AluOpType.max)

                # --- windows of this block: quads share one psum bank ---
                for q0 in range(0, BL, 4):
                    for h in range(2):
                        ps4 = psmt.tile([128, 4, 128], F32, name="ps4", tag="pt")
                        first = True
                        for qi in range(4):
                            nb = q0 + qi
                            np_ = base_n + nb
                            off = np_ + 1
                            for c in DVE_CHUNKS:
                                if c // 16 != h:
                                    continue
                                ab = a16[c][:]
                                mov = AP(ab.tensor, ab.offset + nb,
                                         [[ab.ap[0][0], 128], [BL, W]])
                                nc.tensor.matmul(
                                    ps4[:, qi, :], e_all[:, c % NE, :], mov,
                                    start=first, stop=False,
                                    skip_group_check=True)
                                first = False
                            for s, (c0, c1) in enumerate(DR_PAIRS):
                                if c0 // 16 != h:
                                    continue
                                ab = a8p[s][:]
                                kst = W * BL if c1 is not None else 0
                                mov = AP(ab.tensor, ab.offset + nb,
                                         [[ab.ap[0][0], 128], [kst, 2], [BL, W]])
                                nc.tensor.matmul(
                                    ps4[:, qi, :], e8_all[:, s], mov,
                                    start=first, stop=False, perf_mode=DR,
                                    skip_group_check=True)
                                first = False
                            nc.tensor.matmul(
                                ps4[:, qi, :], e_all[:, NE, :],
                                smnegd[:, h, off : off + W],
                                start=False,
                                stop=(qi == 3),
                                skip_group_check=True)
                        # one batched exp for 4 windows (no bias)
                        np0 = base_n + q0
                        nc.scalar.activation(
                            out=esc_all[h][:, np0 : np0 + 4, :],
                            in_=ps4[:],
                            func=mybir.ActivationFunctionType.Exp,
                            scale=-1.0)
                        # per-window rescale by e^{SMdiff_n} + row sums
                        for qi in range(4):
                            np_ = np0 + qi
                            off = np_ + 1
                            nc.vector.tensor_scalar(
                                out=esc_all[h][:, np_, :],
                                in0=esc_all[h][:, np_, :],
                                scalar1=sexp[:, h, np_ : np_ + 1],
                                scalar2=None,
                                op0=mybir.AluOpType.mult,
                                op1=mybir.AluOpType.add,
                                accum_out=row_acc[:, h, np_ : np_ + 1])
                            nc.tensor.matmul(
                                col_ps[h][:, off : off + W], e_all[:, NE, :],
                                esc_all[h][:, np_, :],
                                start=False, stop=(np_ == NPER - 1),
                                skip_group_check=True)
                base_n += BL

            # ---------------- outputs ----------------
            for h in range(2):
                col_sb = wpool.tile([128, JW], F32, name=f"col_sb{h}")
                nc.scalar.copy(out=col_sb[:], in_=col_ps[h][:])
                nc.sync.dma_start(col_d[h], col_sb[:])
                nc.sync.dma_start(row_d[h], row_acc[:, h, :])
                # dup = k=128 column of each window
                nc.sync.dma_start(dup_d[h], esc_all[h][:, :, W - 1])

    nc.finalize()
    return nc


def _get_compiled():
    global _compiled
    if _compiled is None:
        _compiled = _build()
    return _compiled


def _prep_inputs(x, T):
    """Per-core input maps. Core c gets x row-rotated by -NPER*c."""
    import ml_dtypes

    f16 = np.float16
    f8 = ml_dtypes.float8_e4m3fn
    wn = np.ascontiguousarray(T.reshape(F, B * I)).astype(f16)
    wsum = T.reshape(F, B, I).sum(axis=2).astype(f16)
    # Wsumdiff: +wsum for X-form chunks' b-ranges, -wsum for R-form (ACT)
    sign = np.ones((B,), np.float32)
    for c in R_FORM_CHUNKS:
        sign[8 * c : 8 * c + 8] = -1.0
    wsumdiff = (wsum.astype(np.float32) * sign[None, :]).astype(f16)
    e16 = np.zeros((NE + 1, 128, 128), dtype=f16)
    for ei in range(NE):
        for p in range(128):
            e16[ei, p, 8 * ei + p // 16] = 2.0
    e16[NE] = np.eye(128, dtype=f16)
    e8 = np.zeros((len(DR_PAIRS), 128, 2, 128), dtype=f8)
    for s, pair in enumerate(DR_PAIRS):
        for t, c in enumerate(pair):
            if c is None:
                continue
            ei = c % NE
            for p in range(128):
                e8[s, p, t, 8 * ei + p // 16] = 2.0
    in_maps = []
    for c in range(NCORES):
        xr = np.roll(x, -NPER * c, axis=0)
        xT = np.ascontiguousarray(xr.T[:, 0:JW]).astype(f16)
        in_maps.append({
            "xT": xT, "Wn": wn, "Wsum": wsum, "Wsumdiff": wsumdiff,
            "E16": e16, "E8": e8,
        })
    return in_maps


def _assemble(x, results):
    """Combine symmetric-pair partials (same scheme as v2)."""
    out_disc = np.zeros((N, B), dtype=np.float32)
    for c, res in enumerate(results):
        rows = res["out_row"].transpose(2, 0, 1).reshape(NPER, B)
        dups = res["out_dup"].astype(np.float32).transpose(2, 0, 1).reshape(NPER, B)
        out_disc[NPER * c : NPER * (c + 1), :] += rows - dups
        colg = np.zeros((B, N), np.float32)
        colg[0:128, 0:JW] = res["out_col"][0]
        colg[128:256, 0:JW] = res["out_col"][1]
        out_disc += np.roll(colg.T, NPER * c, axis=0)
    return np.concatenate([x.astype(np.float32), out_disc], axis=1)


def kernel_run(x, T, trace=False):
    from concourse.bass_utils import run_bass_kernel_spmd

    nc = _get_compiled()
    in_maps = _prep_inputs(x, T)
    res = run_bass_kernel_spmd(nc, in_maps, core_ids=list(range(NCORES)), trace=trace)
    return _assemble(x, res.results), res


def kernel(x, T):
    out, _ = kernel_run(x, T, trace=False)
    return out
